# revision 1
# baseline (speedup 1.0000x reference)
"""LocallyConnected2d (64,64,32,32) x (1,64,64,32,32,9) -> (64,64,32,32) on 8 trn2 cores.

Strategy: x-stationary dataflow
--------------------------------
Spatial sharding over output rows: core i computes output rows [4i, 4i+4).

Unlike the weight-stationary formulation (stationary = per-location weights,
reused over only 64 batch columns -> LDWEIGHTS-bound), here the PE stationary
is an x-patch tile S[r,u] = [K=128, M=64]: partitions (j,c) hold input
xp[c, band_row r, w=2u+j, batch], i.e. channels x two adjacent padded-width
columns. Each stationary is reused by every (output location, tap) that reads
those two input columns -- up to 12 weight-moving matmuls x 64 cols -- so the
weights stream through the PE as the moving operand, read once from SBUF,
while LDWEIGHTS traffic drops ~6x.

Output locations y in [0,16) run on PE column-group h0 (psum partitions 0:64),
y in [16,32) on h64 (psum partitions 64:128); the two halves execute
concurrently on the two halves of the 128-wide PE array. Joint round t
processes input w-pair u=t for the left half and u=t+8 for the right half.

Per (round t, half, band row rb) with valid output rows xs = {rb-2,rb-1,rb}:
  full1: K=128 matmul, y_loc=2t-1 (taps kw=1,2)       -> bank t-1 phase 1
  full0: K=128 matmul, y_loc=2t   (taps kw=0,1)       -> bank t   phase 0
  ej0:   K=64  (parts 0:64),  y_loc=2t-2 (tap kw=2)   -> bank t-1 phase 0
  ej1:   K=64  (parts 64:128),y_loc=2t+1 (tap kw=0)   -> bank t   phase 1
PSUM bank t = y-pair {2t, 2t+1} (per half), col = phase*256 + x*64 + o;
zero-initialized by vector-engine memsets (bias is added on the host during
unpack), drained to SBUF fp16 by the vector engine after round t+1, out-DMAs
deferred to the end of the stream. Within a round, all full-width matmuls are
batched before all quadrant (edge) matmuls: a quadrant LDWEIGHTS cannot
preload during a full-width stream, so each full<->quadrant transition costs
~145ns -- batching pays it once per round instead of once per rb-group.

Compute dtype fp16 (fp32 accumulate in PSUM); output fp16. The kernel is
DMA-bound: ~9.8MB weights + 1.8MB x in, 1MB out per core.
"""

import numpy as np

N_B, C, H, W_W, O = 64, 64, 32, 32, 64
NCORES = 8
RPC = H // NCORES              # 4 output rows per core
BAND = RPC + 2                 # 6 padded input rows per core
NT = 9                         # joint rounds
NBANK = 8                      # psum banks = y-pairs per half
XCOLS = NT * 768               # x sbuf cols: 9 slots x (2 halves x 6 rb x 64 b)
BIAS_COLS = NBANK * 512

COMPUTE_NP = np.float16

_CACHE = {}


def _round_blocks(t):
    """Which blocks exist at joint round t (same for both halves)."""
    return {
        "full1": t >= 1,   # y_loc = 2t-1, kw in {1,2}, K=128, bank t-1, phase 1
        "full0": t <= 7,   # y_loc = 2t,   kw in {0,1}, K=128, bank t,   phase 0
        "ej0": t >= 1,     # y_loc = 2t-2, kw = 2, K=64 parts 0:64,  bank t-1, ph 0
        "ej1": t <= 7,     # y_loc = 2t+1, kw = 0, K=64 parts 64:128, bank t, ph 1
    }


def _xs_for(rb):
    return [x for x in (rb - 2, rb - 1, rb) if 0 <= x <= RPC - 1]


def w_layout():
    """Weight SBUF/DRAM column layout.

    Returns (total_cols, round_col_start, blocks) where blocks maps
    (t, half, rb) -> dict(kind -> (col_off, nx, xmin)). Edge j0/j1 share one
    column slot (j0 on partitions 0:64, j1 on 64:128).
    """
    blocks = {}
    col = 0
    round_start = []
    for t in range(NT):
        round_start.append(col)
        present = _round_blocks(t)
        for half in range(2):
            for rb in range(BAND):
                xs = _xs_for(rb)
                if not xs:
                    continue
                nx = len(xs)
                ent = {}
                if present["full1"]:
                    ent["full1"] = (col, nx, xs[0])
                    col += nx * 64
                if present["full0"]:
                    ent["full0"] = (col, nx, xs[0])
                    col += nx * 64
                # shared edge slot
                ent["edge"] = (col, nx, xs[0])
                col += nx * 64
                blocks[(t, half, rb)] = ent
    round_start.append(col)
    return col, round_start, blocks


W_COLS, W_ROUND_START, W_BLOCKS = w_layout()


def _mybir_dt(np_dt):
    import concourse.mybir as mybir
    import ml_dtypes

    if np_dt == np.float16:
        return mybir.dt.float16
    if np_dt == np.float32:
        return mybir.dt.float32
    if np_dt == ml_dtypes.bfloat16:
        return mybir.dt.bfloat16
    raise ValueError(np_dt)


def build_nc(compute_np=None):
    """Build the (single-program) Bass kernel; same NEFF runs on all 8 cores."""
    import concourse.bass as bass  # noqa: F401
    import concourse.mybir as mybir
    import concourse.tile as tile
    from concourse import bacc
    from contextlib import ExitStack

    cdt = _mybir_dt(compute_np or COMPUTE_NP)
    f32 = mybir.dt.float32

    nc = bacc.Bacc("TRN2", target_bir_lowering=False, debug=False)

    x_dram = nc.dram_tensor("xb", [128, XCOLS], cdt, kind="ExternalInput")
    w_dram = nc.dram_tensor("wp", [128, W_COLS], cdt, kind="ExternalInput")
    o_dram = nc.dram_tensor("out", [NBANK, 128, 512], cdt, kind="ExternalOutput")

    with ExitStack() as ctx:
        tc = ctx.enter_context(tile.TileContext(nc))
        const = ctx.enter_context(tc.tile_pool(name="const", bufs=1))
        wpool = ctx.enter_context(tc.tile_pool(name="wpool", bufs=NT))
        ppool = ctx.enter_context(tc.tile_pool(name="ppool", bufs=1, space="PSUM"))
        spool = ctx.enter_context(tc.tile_pool(name="spool", bufs=NBANK))

        xsb = const.tile([128, XCOLS], cdt)

        # x slot 0 alone first on scalar (smallest round-0 prerequisite);
        # later slots stream on gpsimd in need-order chunks. Weight-chunk
        # rows stay large: sub-9KB DMA rows are packet-overhead-bound.
        nc.scalar.dma_start(xsb[:, 0:768], x_dram.ap()[:, 0:768])
        nc.gpsimd.dma_start(xsb[:, 768 : 3 * 768], x_dram.ap()[:, 768 : 3 * 768])
        nc.gpsimd.dma_start(xsb[:, 3 * 768 : 6 * 768], x_dram.ap()[:, 3 * 768 : 6 * 768])
        nc.gpsimd.dma_start(xsb[:, 6 * 768 : XCOLS], x_dram.ap()[:, 6 * 768 : XCOLS])

        banks = [ppool.tile([128, 512], f32, name=f"bank{b}") for b in range(NBANK)]
        for b in range(NBANK):
            nc.vector.memset(banks[b][:, :], 0.0)
        wt = [None] * NT

        WTILE = max(W_ROUND_START[t + 1] - W_ROUND_START[t] for t in range(NT))
        for t in range(NT):
            # weight chunk for this round; alternate issuing queue
            c0, c1 = W_ROUND_START[t], W_ROUND_START[t + 1]
            wt[t] = wpool.tile([128, WTILE], cdt, name="wt")
            # weights ride the two fast queues, interleaved by need-time;
            # wt1 is split across both so round 1 starts at supply minimum
            if t == 1:
                h = (c1 - c0) // 2
                nc.sync.dma_start(wt[t][:, 0:h], w_dram.ap()[:, c0 : c0 + h])
                nc.scalar.dma_start(wt[t][:, h : c1 - c0], w_dram.ap()[:, c0 + h : c1])
            else:
                qmap = {0: nc.sync, 2: nc.sync, 4: nc.sync, 6: nc.sync, 8: nc.sync,
                        3: nc.scalar, 5: nc.scalar, 7: nc.scalar}
                qmap[t].dma_start(wt[t][:, 0 : c1 - c0], w_dram.ap()[:, c0:c1])

        stages = []
        for t in range(NT):
            c0 = W_ROUND_START[t]
            present = _round_blocks(t)
            def _hc(rb):
                out = []
                for half in range(2):
                    ent = W_BLOCKS[(t, half, rb)]
                    S = xsb[
                        :,
                        t * 768 + half * 384 + rb * 64 : t * 768 + half * 384 + rb * 64 + 64,
                    ]
                    out.append((ent, S, half * 64))
                return out

            rbs = [rb for rb in range(BAND) if (t, 0, rb) in W_BLOCKS]
            # Phase A: all full-width (K=128) matmuls of the round. These
            # pipeline back-to-back; h0/h64 col-groups run in lockstep.
            for rb in rbs:
                hc = _hc(rb)
                if present["full1"]:
                    for ent, S, p0 in hc:
                        off, nx, xmin = ent["full1"]
                        nc.tensor.matmul(
                            banks[t - 1][p0 : p0 + 64, 256 + xmin * 64 : 256 + (xmin + nx) * 64],
                            S,
                            wt[t][:, off - c0 : off - c0 + nx * 64],
                            start=False,
                            stop=False,
                            skip_group_check=True,
                        )
                if present["full0"]:
                    for ent, S, p0 in hc:
                        off, nx, xmin = ent["full0"]
                        nc.tensor.matmul(
                            banks[t][p0 : p0 + 64, xmin * 64 : (xmin + nx) * 64],
                            S,
                            wt[t][:, off - c0 : off - c0 + nx * 64],
                            start=False,
                            stop=False,
                            skip_group_check=True,
                        )
            # Phase B: all edge (64-row quadrant) matmuls. Batched so the
            # expensive full<->quadrant LDWEIGHTS transition happens once per
            # round instead of once per rb-group.
            for rb in rbs:
                hc = _hc(rb)

                def _ej0(hx):
                    ent, S, p0 = hc[hx]
                    off, nx, xmin = ent["edge"]
                    nc.tensor.matmul(
                        banks[t - 1][p0 : p0 + 64, xmin * 64 : (xmin + nx) * 64],
                        S[0:64, :],
                        wt[t][0:64, off - c0 : off - c0 + nx * 64],
                        start=False,
                        stop=(rb == BAND - 1),
                        skip_group_check=True,
                    )

                def _ej1(hx):
                    ent, S, p0 = hc[hx]
                    off, nx, xmin = ent["edge"]
                    nc.tensor.matmul(
                        banks[t][p0 : p0 + 64, 256 + xmin * 64 : 256 + (xmin + nx) * 64],
                        S[64:128, :],
                        wt[t][64:128, off - c0 : off - c0 + nx * 64],
                        start=False,
                        stop=(t == NT - 1 and rb == BAND - 1),
                        skip_group_check=True,
                    )

                if present["ej0"] and present["ej1"]:
                    _ej0(0)
                    _ej1(1)
                    _ej1(0)
                    _ej0(1)
                elif present["ej0"]:
                    _ej0(0)
                    _ej0(1)
                elif present["ej1"]:
                    _ej1(0)
                    _ej1(1)
            # drain bank t-1 (its last write was this round); out-DMAs are
            # deferred to the end so they don't steal inbound HBM bandwidth
            if t >= 1:
                stg = spool.tile([128, 512], cdt, name="stg")
                nc.vector.tensor_copy(stg[:], banks[t - 1][:, :])
                stages.append(stg)

        for b, stg in enumerate(stages):
            oeng = (nc.gpsimd, nc.scalar, nc.sync)[b % 3]
            oeng.dma_start(o_dram.ap()[b], stg[:])

    nc.compile()
    return nc


def pack_inputs(x, weight, bias, compute_np=None):
    """Full fp32 inputs -> list of 8 per-core input dicts (device layouts)."""
    cnp = compute_np or COMPUTE_NP
    x = np.asarray(x)
    w5 = np.asarray(weight)[0]        # (o, c, X, Y, 9)  k = kh*3 + kw
    b3 = np.asarray(bias)[0]          # (o, X, Y)

    xp = np.pad(x, ((0, 0), (0, 0), (1, 1), (1, 1))).astype(cnp)  # (b, c, 34, 34)

    # w column index for (t, half) -> w-col = 2t + 16*half + j
    in_maps = []
    for i in range(NCORES):
        band = xp[:, :, RPC * i : RPC * i + BAND, :]          # (b, c, 6, 34)
        # xb[j*64+c, t*768 + half*384 + rb*64 + b]
        xb = np.zeros((2, 64, XCOLS // 64, 64), dtype=cnp)    # (j, c, slotcol, b)
        for t in range(NT):
            for half in range(2):
                for j in range(2):
                    w_col = 2 * t + 16 * half + j
                    # band[b, c, rb, w_col] -> (j, c, rb, b)
                    blk = band[:, :, :, w_col].transpose(1, 2, 0)  # (c, rb, b)
                    for rb in range(BAND):
                        xb[j, :, (t * 768 + half * 384) // 64 + rb, :] = blk[:, rb, :]
        xb = xb.reshape(128, XCOLS)

        wp = np.zeros((128, W_COLS), dtype=cnp)
        wc = w5[:, :, RPC * i : RPC * (i + 1), :, :]          # (o, c, 4, 32, 9)
        for (t, half, rb), ent in W_BLOCKS.items():
            xs = _xs_for(rb)
            for kind, (off, nx, xmin) in ent.items():
                for xi, xx in enumerate(xs):
                    kh = rb - xx
                    cols = slice(off + xi * 64, off + (xi + 1) * 64)
                    if kind == "full1":
                        y = 16 * half + 2 * t - 1
                        # rows (j,c): kw = 1 + j
                        blk = wc[:, :, xx, y, :]
                        wp[0:64, cols] = blk[:, :, 3 * kh + 1].T
                        wp[64:128, cols] = blk[:, :, 3 * kh + 2].T
                    elif kind == "full0":
                        y = 16 * half + 2 * t
                        blk = wc[:, :, xx, y, :]
                        wp[0:64, cols] = blk[:, :, 3 * kh + 0].T
                        wp[64:128, cols] = blk[:, :, 3 * kh + 1].T
                    else:  # edge slot
                        if t >= 1:
                            y = 16 * half + 2 * t - 2
                            wp[0:64, cols] = wc[:, :, xx, y, 3 * kh + 2].T
                        if t <= 7:
                            y = 16 * half + 2 * t + 1
                            wp[64:128, cols] = wc[:, :, xx, y, 3 * kh + 0].T

        in_maps.append({"xb": xb, "wp": wp})
    return in_maps


def unpack_output(core_outs, bias):
    """8 per-core [NBANK,128,512] fp16 arrays -> full (64, 64, 32, 32) output.

    Bias is added on the host (it is a per-(o,x,y) constant broadcast over
    batch, cheaper here than on-device psum init).
    """
    ar = np.stack(core_outs).astype(np.float32)   # (core, t, p, col)
    ar = ar.reshape(8, 8, 2, 64, 2, 4, 64)        # core t half b ph x o
    out = ar.transpose(3, 6, 0, 5, 2, 1, 4)       # b o core x half t ph
    out = np.ascontiguousarray(out.reshape(64, 64, 32, 32), dtype=np.float32)
    out += np.asarray(bias, dtype=np.float32)[0][None, :, :, :]
    return out


def run_on_device(in_maps, trace=False, compute_np=None, **kwargs):
    from concourse import bass_utils

    key = ("nc", np.dtype(compute_np or COMPUTE_NP).name)
    if key not in _CACHE:
        _CACHE[key] = build_nc(compute_np)
    nc = _CACHE[key]
    res = bass_utils.run_bass_kernel_spmd(
        nc, in_maps, core_ids=list(range(NCORES)), trace=trace, **kwargs
    )
    return res


def kernel(x, weight, bias):
    in_maps = pack_inputs(x, weight, bias)
    res = run_on_device(in_maps)
    return unpack_output([r["out"] for r in res.results], bias)



# revision 10
# speedup vs baseline: 1.3341x; 1.3341x over previous
"""LocallyConnected2d (64,64,32,32) x (1,64,64,32,32,9) -> (64,64,32,32) on 8 trn2 cores.

Strategy: x-stationary dataflow
--------------------------------
Spatial sharding over output rows: core i computes output rows [4i, 4i+4).

Unlike the weight-stationary formulation (stationary = per-location weights,
reused over only 64 batch columns -> LDWEIGHTS-bound), here the PE stationary
is an x-patch tile S[r,u] = [K=128, M=64]: partitions (j,c) hold input
xp[c, band_row r, w=2u+j, batch], i.e. channels x two adjacent padded-width
columns. Each stationary is reused by every (output location, tap) that reads
those two input columns -- up to 12 weight-moving matmuls x 64 cols -- so the
weights stream through the PE as the moving operand, read once from SBUF,
while LDWEIGHTS traffic drops ~6x.

Output locations y in [0,16) run on PE column-group h0 (psum partitions 0:64),
y in [16,32) on h64 (psum partitions 64:128); the two halves execute
concurrently on the two halves of the 128-wide PE array. Joint round t
processes input w-pair u=t for the left half and u=t+8 for the right half.

Per (round t, half, band row rb) with valid output rows xs = {rb-2,rb-1,rb}:
  full1: K=128 matmul, y_loc=2t-1 (taps kw=1,2)       -> bank t-1 phase 1
  full0: K=128 matmul, y_loc=2t   (taps kw=0,1)       -> bank t   phase 0
  ej0:   K=64  (parts 0:64),  y_loc=2t-2 (tap kw=2)   -> bank t-1 phase 0
  ej1:   K=64  (parts 64:128),y_loc=2t+1 (tap kw=0)   -> bank t   phase 1
PSUM bank t = y-pair {2t, 2t+1} (per half), col = phase*256 + x*64 + o;
zero-initialized by vector-engine memsets (bias is added on the host during
unpack), drained to SBUF fp16 by the vector engine after round t+1, out-DMAs
deferred to the end of the stream. Within a round, all full-width matmuls are
batched before all quadrant (edge) matmuls: a quadrant LDWEIGHTS cannot
preload during a full-width stream, so each full<->quadrant transition costs
~145ns -- batching pays it once per round instead of once per rb-group.

Mixed precision: x (stationary) fp16, weights (moving) fp8 E3M4 — the PE
allows different operand dtypes, PSUM accumulates fp32. E3M4 weight
quantization gives ~1.34e-2 output rel err (vs 2e-2 budget) while halving
the dominant weight stream: ~4.7MB weights + 1.8MB x in, 1MB out per core.
At ~368 GB/s of DMA-engine bandwidth that is ~20.5us of DMA, balancing the
~20.5us of PE column-streaming — the ridge point.
"""

import numpy as np
import ml_dtypes

N_B, C, H, W_W, O = 64, 64, 32, 32, 64
NCORES = 8
RPC = H // NCORES              # 4 output rows per core
BAND = RPC + 2                 # 6 padded input rows per core
NT = 9                         # joint rounds
NBANK = 8                      # psum banks = y-pairs per half
XCOLS = NT * 768               # x sbuf cols: 9 slots x (2 halves x 6 rb x 64 b)
BIAS_COLS = NBANK * 512

X_NP = np.float16              # stationary / activations
W_NP = ml_dtypes.float8_e3m4   # moving / weights
OUT_NP = np.float16

_CACHE = {}


def _round_blocks(t):
    """Which blocks exist at joint round t (same for both halves)."""
    return {
        "full1": t >= 1,   # y_loc = 2t-1, kw in {1,2}, K=128, bank t-1, phase 1
        "full0": t <= 7,   # y_loc = 2t,   kw in {0,1}, K=128, bank t,   phase 0
        "ej0": t >= 1,     # y_loc = 2t-2, kw = 2, K=64 parts 0:64,  bank t-1, ph 0
        "ej1": t <= 7,     # y_loc = 2t+1, kw = 0, K=64 parts 64:128, bank t, ph 1
    }


def _xs_for(rb):
    return [x for x in (rb - 2, rb - 1, rb) if 0 <= x <= RPC - 1]


def w_layout():
    """Weight SBUF/DRAM column layout.

    Returns (total_cols, round_col_start, blocks) where blocks maps
    (t, half, rb) -> dict(kind -> (col_off, nx, xmin)). Edge j0/j1 share one
    column slot (j0 on partitions 0:64, j1 on 64:128).
    """
    blocks = {}
    col = 0
    round_start = []
    for t in range(NT):
        round_start.append(col)
        present = _round_blocks(t)
        for half in range(2):
            for rb in range(BAND):
                xs = _xs_for(rb)
                if not xs:
                    continue
                nx = len(xs)
                ent = {}
                if present["full1"]:
                    ent["full1"] = (col, nx, xs[0])
                    col += nx * 64
                if present["full0"]:
                    ent["full0"] = (col, nx, xs[0])
                    col += nx * 64
                # shared edge slot
                ent["edge"] = (col, nx, xs[0])
                col += nx * 64
                blocks[(t, half, rb)] = ent
    round_start.append(col)
    return col, round_start, blocks


W_COLS, W_ROUND_START, W_BLOCKS = w_layout()


def _mybir_dt(np_dt):
    import concourse.mybir as mybir

    if np_dt == np.float16:
        return mybir.dt.float16
    if np_dt == np.float32:
        return mybir.dt.float32
    if np_dt == ml_dtypes.bfloat16:
        return mybir.dt.bfloat16
    if np_dt == ml_dtypes.float8_e3m4:
        return mybir.dt.float8e3
    raise ValueError(np_dt)


def build_nc(compute_np=None):
    """Build the (single-program) Bass kernel; same NEFF runs on all 8 cores."""
    import concourse.bass as bass  # noqa: F401
    import concourse.mybir as mybir
    import concourse.tile as tile
    from concourse import bacc
    from contextlib import ExitStack

    xdt = _mybir_dt(X_NP)
    wdt = _mybir_dt(W_NP)
    odt = _mybir_dt(OUT_NP)
    f32 = mybir.dt.float32

    nc = bacc.Bacc("TRN2", target_bir_lowering=False, debug=False)

    x_dram = nc.dram_tensor("xb", [128, XCOLS], xdt, kind="ExternalInput")
    w_dram = nc.dram_tensor("wp", [128, W_COLS], wdt, kind="ExternalInput")
    o_dram = nc.dram_tensor("out", [NBANK, 128, 512], odt, kind="ExternalOutput")

    with ExitStack() as ctx:
        tc = ctx.enter_context(tile.TileContext(nc))
        const = ctx.enter_context(tc.tile_pool(name="const", bufs=1))
        wpool = ctx.enter_context(tc.tile_pool(name="wpool", bufs=NT))
        ppool = ctx.enter_context(tc.tile_pool(name="ppool", bufs=1, space="PSUM"))
        spool = ctx.enter_context(tc.tile_pool(name="spool", bufs=NBANK))

        xsb = const.tile([128, XCOLS], xdt)

        # x slot 0 alone first on scalar (smallest round-0 prerequisite);
        # later slots stream on gpsimd in need-order chunks. Weight-chunk
        # rows stay large: sub-9KB DMA rows are packet-overhead-bound.
        nc.scalar.dma_start(xsb[:, 0:768], x_dram.ap()[:, 0:768])
        nc.gpsimd.dma_start(xsb[:, 768 : 3 * 768], x_dram.ap()[:, 768 : 3 * 768])
        nc.gpsimd.dma_start(xsb[:, 3 * 768 : 6 * 768], x_dram.ap()[:, 3 * 768 : 6 * 768])
        nc.gpsimd.dma_start(xsb[:, 6 * 768 : XCOLS], x_dram.ap()[:, 6 * 768 : XCOLS])

        banks = [ppool.tile([128, 512], f32, name=f"bank{b}") for b in range(NBANK)]
        for b in range(NBANK):
            nc.vector.memset(banks[b][:, :], 0.0)
        wt = [None] * NT

        WTILE = max(W_ROUND_START[t + 1] - W_ROUND_START[t] for t in range(NT))
        for t in range(NT):
            # weight chunk for this round; alternate issuing queue
            c0, c1 = W_ROUND_START[t], W_ROUND_START[t + 1]
            wt[t] = wpool.tile([128, WTILE], wdt, name="wt")
            # weights ride the two fast queues, interleaved by need-time;
            # wt1 is split across both so round 1 starts at supply minimum
            if t == 1:
                h = (c1 - c0) // 2
                nc.sync.dma_start(wt[t][:, 0:h], w_dram.ap()[:, c0 : c0 + h])
                nc.scalar.dma_start(wt[t][:, h : c1 - c0], w_dram.ap()[:, c0 + h : c1])
            else:
                qmap = {0: nc.sync, 2: nc.sync, 4: nc.sync, 6: nc.sync, 8: nc.sync,
                        3: nc.scalar, 5: nc.scalar, 7: nc.scalar}
                qmap[t].dma_start(wt[t][:, 0 : c1 - c0], w_dram.ap()[:, c0:c1])

        stages = []
        for t in range(NT):
            c0 = W_ROUND_START[t]
            present = _round_blocks(t)
            def _hc(rb):
                out = []
                for half in range(2):
                    ent = W_BLOCKS[(t, half, rb)]
                    S = xsb[
                        :,
                        t * 768 + half * 384 + rb * 64 : t * 768 + half * 384 + rb * 64 + 64,
                    ]
                    out.append((ent, S, half * 64))
                return out

            rbs = [rb for rb in range(BAND) if (t, 0, rb) in W_BLOCKS]
            # Phase A: all full-width (K=128) matmuls of the round. These
            # pipeline back-to-back; h0/h64 col-groups run in lockstep.
            for rb in rbs:
                hc = _hc(rb)
                if present["full1"]:
                    for ent, S, p0 in hc:
                        off, nx, xmin = ent["full1"]
                        nc.tensor.matmul(
                            banks[t - 1][p0 : p0 + 64, 256 + xmin * 64 : 256 + (xmin + nx) * 64],
                            S,
                            wt[t][:, off - c0 : off - c0 + nx * 64],
                            start=False,
                            stop=False,
                            skip_group_check=True,
                        )
                if present["full0"]:
                    for ent, S, p0 in hc:
                        off, nx, xmin = ent["full0"]
                        nc.tensor.matmul(
                            banks[t][p0 : p0 + 64, xmin * 64 : (xmin + nx) * 64],
                            S,
                            wt[t][:, off - c0 : off - c0 + nx * 64],
                            start=False,
                            stop=False,
                            skip_group_check=True,
                        )
            # Phase B: all edge (64-row quadrant) matmuls. Batched so the
            # expensive full<->quadrant LDWEIGHTS transition happens once per
            # round instead of once per rb-group.
            for rb in rbs:
                hc = _hc(rb)

                def _ej0(hx):
                    ent, S, p0 = hc[hx]
                    off, nx, xmin = ent["edge"]
                    nc.tensor.matmul(
                        banks[t - 1][p0 : p0 + 64, xmin * 64 : (xmin + nx) * 64],
                        S[0:64, :],
                        wt[t][0:64, off - c0 : off - c0 + nx * 64],
                        start=False,
                        stop=(rb == BAND - 1),
                        skip_group_check=True,
                    )

                def _ej1(hx):
                    ent, S, p0 = hc[hx]
                    off, nx, xmin = ent["edge"]
                    nc.tensor.matmul(
                        banks[t][p0 : p0 + 64, 256 + xmin * 64 : 256 + (xmin + nx) * 64],
                        S[64:128, :],
                        wt[t][64:128, off - c0 : off - c0 + nx * 64],
                        start=False,
                        stop=(t == NT - 1 and rb == BAND - 1),
                        skip_group_check=True,
                    )

                if present["ej0"] and present["ej1"]:
                    _ej0(0)
                    _ej1(1)
                    _ej1(0)
                    _ej0(1)
                elif present["ej0"]:
                    _ej0(0)
                    _ej0(1)
                elif present["ej1"]:
                    _ej1(0)
                    _ej1(1)
            # drain bank t-1 (its last write was this round); out-DMAs are
            # deferred to the end so they don't steal inbound HBM bandwidth
            if t >= 1:
                stg = spool.tile([128, 512], odt, name="stg")
                nc.vector.tensor_copy(stg[:], banks[t - 1][:, :])
                stages.append(stg)

        for b, stg in enumerate(stages):
            oeng = (nc.gpsimd, nc.scalar, nc.sync)[b % 3]
            oeng.dma_start(o_dram.ap()[b], stg[:])

    nc.compile()
    return nc


def pack_inputs(x, weight, bias, compute_np=None):
    """Full fp32 inputs -> list of 8 per-core input dicts (device layouts)."""
    x = np.asarray(x)
    w5 = np.asarray(weight)[0]        # (o, c, X, Y, 9)  k = kh*3 + kw
    b3 = np.asarray(bias)[0]          # (o, X, Y)

    xp = np.pad(x, ((0, 0), (0, 0), (1, 1), (1, 1))).astype(X_NP)  # (b, c, 34, 34)

    # w column index for (t, half) -> w-col = 2t + 16*half + j
    in_maps = []
    for i in range(NCORES):
        band = xp[:, :, RPC * i : RPC * i + BAND, :]          # (b, c, 6, 34)
        # xb[j*64+c, t*768 + half*384 + rb*64 + b]
        xb = np.zeros((2, 64, XCOLS // 64, 64), dtype=X_NP)   # (j, c, slotcol, b)
        for t in range(NT):
            for half in range(2):
                for j in range(2):
                    w_col = 2 * t + 16 * half + j
                    # band[b, c, rb, w_col] -> (j, c, rb, b)
                    blk = band[:, :, :, w_col].transpose(1, 2, 0)  # (c, rb, b)
                    for rb in range(BAND):
                        xb[j, :, (t * 768 + half * 384) // 64 + rb, :] = blk[:, rb, :]
        xb = xb.reshape(128, XCOLS)

        wp = np.zeros((128, W_COLS), dtype=W_NP)
        wc = w5[:, :, RPC * i : RPC * (i + 1), :, :]          # (o, c, 4, 32, 9)
        for (t, half, rb), ent in W_BLOCKS.items():
            xs = _xs_for(rb)
            for kind, (off, nx, xmin) in ent.items():
                for xi, xx in enumerate(xs):
                    kh = rb - xx
                    cols = slice(off + xi * 64, off + (xi + 1) * 64)
                    if kind == "full1":
                        y = 16 * half + 2 * t - 1
                        # rows (j,c): kw = 1 + j
                        blk = wc[:, :, xx, y, :]
                        wp[0:64, cols] = blk[:, :, 3 * kh + 1].T
                        wp[64:128, cols] = blk[:, :, 3 * kh + 2].T
                    elif kind == "full0":
                        y = 16 * half + 2 * t
                        blk = wc[:, :, xx, y, :]
                        wp[0:64, cols] = blk[:, :, 3 * kh + 0].T
                        wp[64:128, cols] = blk[:, :, 3 * kh + 1].T
                    else:  # edge slot
                        if t >= 1:
                            y = 16 * half + 2 * t - 2
                            wp[0:64, cols] = wc[:, :, xx, y, 3 * kh + 2].T
                        if t <= 7:
                            y = 16 * half + 2 * t + 1
                            wp[64:128, cols] = wc[:, :, xx, y, 3 * kh + 0].T

        in_maps.append({"xb": xb, "wp": wp})
    return in_maps


def unpack_output(core_outs, bias):
    """8 per-core [NBANK,128,512] fp16 arrays -> full (64, 64, 32, 32) output.

    Bias is added on the host (it is a per-(o,x,y) constant broadcast over
    batch, cheaper here than on-device psum init).
    """
    ar = np.stack(core_outs).astype(np.float32)   # (core, t, p, col)
    ar = ar.reshape(8, 8, 2, 64, 2, 4, 64)        # core t half b ph x o
    out = ar.transpose(3, 6, 0, 5, 2, 1, 4)       # b o core x half t ph
    out = np.ascontiguousarray(out.reshape(64, 64, 32, 32), dtype=np.float32)
    out += np.asarray(bias, dtype=np.float32)[0][None, :, :, :]
    return out


def run_on_device(in_maps, trace=False, compute_np=None, **kwargs):
    from concourse import bass_utils

    key = "nc"
    if key not in _CACHE:
        _CACHE[key] = build_nc(compute_np)
    nc = _CACHE[key]
    res = bass_utils.run_bass_kernel_spmd(
        nc, in_maps, core_ids=list(range(NCORES)), trace=trace, **kwargs
    )
    return res


def kernel(x, weight, bias):
    in_maps = pack_inputs(x, weight, bias)
    res = run_on_device(in_maps)
    return unpack_output([r["out"] for r in res.results], bias)

